# revision 64
# baseline (speedup 1.0000x reference)
"""Trainium2 Bass kernel for nn_ImprintedModel (retrieval_knn).

Computes y[c, b] = max over the 32 proxies p of class c of
    (w1[p] / ||w1[p]||) . (data[b] / ||data[b]||)
for data [4096, 512], w1 [64000, 512] (2000 classes x 32 proxies),
output [2000, 4096] fp32.

Sharding: w1 rows (and hence classes) split across 8 cores (8000 rows =
250 classes per core); data replicated.  Each core computes its 250
output rows for all 4096 batch columns; host concatenates/transposes.

Algorithm (per core):
  Host prep (free wrt device time): l2-normalize data rows and w rows,
  scale by S=16, quantize to fp8 e4m3, and pack both operands
  transposed+interleaved for DoubleRow matmuls:
      x8T[c, kp*2F + i*F + f] = x8[f, kp*256 + i*128 + c]
  so a [128, 2, F] SBUF tile per contraction k-pair kp holds the two
  128-row contraction groups the PE consumes per DoubleRow pass.

  Device: for each batch m-tile (128 rows) and each chunk of 32 classes
  (1024 w rows; tail 26/832), four fp8 DoubleRow matmuls (contraction
  512 = 2 k-pairs x 256, two 512-column groups) accumulate scaled
  scores into a [128, 1024] PSUM tile at 0.5 cycles/row -- 4x the
  bf16/f32r rate, ~110us of PE time total.

  The per-class max over 32 proxies is bound by PSUM-drain legality
  (walrus-verified): GPSIMD/Pool cannot access PSUM and has no max op
  (add/copy only), DMA cannot access PSUM, and a DVE tensor_tensor may
  read at most one PSUM operand.  PSUM is drained by DVE and ACT only;
  Pool is recruited through a host-side transform: chunks j0..j4 store
  each proxy pair as s=(w0+w1)/2, d=(w0-w1)/2 rows, so the pairwise max
  becomes s+|d| -- an ADD.  Per chunk: 'G' (j0-3) ACT copies the s-half
  and writes |d| (Abs), then Pool does the fp16 ADD; 'H' (j4) ACT
  writes |d| and DVE adds it to the PSUM s-half; 'R' (j5-7, raw) one
  DVE tensor_reduce(max) finishes all 32 proxies straight from PSUM
  into the output tile.  Chunks are processed in an interleaved order
  (R/H spaced by Gs) so consecutive PSUM tiles drain on different
  engines; a DVE 2x-mode fp16 max tree finishes classes 0..159 per
  m-tile pair, one pair behind the matmul stream (per-half for the
  last pair to shorten the tail).  Scores are 256x true values (S^2);
  the host divides after the gather.
"""

import numpy as np

# Problem shapes (hardcoded; harness always calls with these).
B = 4096
E = 512
C = 2000
PROXIES = 32
P = C * PROXIES
N_CORES = 8
P_SHARD = P // N_CORES      # 8000 w rows per core
C_SHARD = C // N_CORES      # 250 classes per core
EPS = 1e-12
S = 16.0                    # fp8 pre-quant scale (output is S^2 too big)

PE_TILE = 128
MT = B // PE_TILE           # 32 batch m-tiles
NPR = MT // 2               # 16 m-tile pairs
WARM_PAIRS = 2              # leading pairs run chunk-outer (DMA warmup)
CHUNK = 1024                # w rows per chunk (32 classes)
NCH = (P_SHARD + CHUNK - 1) // CHUNK        # 8 chunks (last 832)

# Legal engine facts on TRN2 (walrus-verified): GPSIMD/Pool cannot
# touch PSUM and has no max op at all (add/copy only); DMA cannot touch
# PSUM; a DVE tensor_tensor may read at most one PSUM operand.  PSUM is
# therefore drained by DVE and ACT only.  To still use Pool, chunks
# j0..j4 are stored TRANSFORMED on the host: each proxy pair (2i,2i+1)
# becomes s=(w0+w1)/2, d=(w0-w1)/2 rows, so the pairwise max is s+|d|
# -- an ADD, which Pool does support.
#  'G' (j0..j3): ACT copies s-half + ACT |d|-half -> Pool fp16 ADD
#  'H' (j4):     ACT |d|-half -> DVE ADD (PSUM s-half + SBUF |d|)
#  'R' (j5..j7): DVE tensor_reduce(max) of the raw 32 proxies straight
#                from PSUM into the output tile
# Chunks are processed in an interleaved order so consecutive PSUM
# tiles are drained by different engines (R/H spaced out by Gs).
SD_JS = (0, 1, 2, 3, 4)             # host-transformed chunks
J_ORDER = (0, 5, 1, 6, 2, 4, 3, 7)
WARM_J_ORDER = J_ORDER
TREE_C = 5 * (CHUNK // PROXIES)     # classes finished by the fp16 tree
SMALLS_PAT = ['D'] * 16


def chunk_eng(j):
    if j < 4:
        return 'G'
    if j == 4:
        return 'H'
    return 'R'


WARMUP = 16                 # PE p-state warmup matmuls (0 to disable)


def build_bass_kernel():
    from concourse import bacc, mybir
    from concourse.tile import TileContext

    f32 = mybir.dt.float32
    f16 = mybir.dt.float16
    f8 = mybir.dt.float8e4
    OP = mybir.AluOpType
    AF = mybir.ActivationFunctionType
    AX = mybir.AxisListType
    PM = mybir.MatmulPerfMode

    nc = bacc.Bacc("TRN2", target_bir_lowering=False, debug=False)
    d8_d = nc.dram_tensor("d8", [PE_TILE, 4 * B], f8, kind="ExternalInput")
    w8_d = nc.dram_tensor("w8", [PE_TILE, 4 * P_SHARD], f8,
                          kind="ExternalInput")
    out_d = nc.dram_tensor("out", [B, C_SHARD], f16, kind="ExternalOutput")

    # chunk column ranges and class counts
    chunks = []
    for j in range(NCH):
        cs = j * CHUNK
        ce = min(cs + CHUNK, P_SHARD)
        chunks.append((cs, ce, (ce - cs) // PROXIES))

    with TileContext(nc) as tc:
        with tc.tile_pool(name="sbuf", bufs=1) as sb, \
             tc.tile_pool(name="mmps", bufs=4, space="PSUM") as psm:

            dt = [sb.tile([PE_TILE, 2, B], f8, tag=f"dt{kp}", name=f"dt{kp}")
                  for kp in range(2)]
            wt = [sb.tile([PE_TILE, 2, P_SHARD], f8, tag=f"wt{kp}",
                          name=f"wt{kp}") for kp in range(2)]

            # ---- input DMAs.  Emission order keeps the startup path
            # short: data columns for the warm pairs, then w chunk by
            # chunk, then the remaining data columns.
            nwarm = WARM_PAIRS * 2 * PE_TILE

            def dma_dt(b0, b1):
                for kp in range(2):
                    src = d8_d[:].rearrange("p (k i b) -> p k i b", k=2, i=2)
                    nc.sync.dma_start(dt[kp][:, :, b0:b1],
                                      src[:, kp, :, b0:b1])

            def dma_wt(j):
                cs, ce, _ = chunks[j]
                for kp in range(2):
                    src = w8_d[:].rearrange("p (k i n) -> p k i n", k=2, i=2)
                    nc.sync.dma_start(wt[kp][:, :, cs:ce],
                                      src[:, kp, :, cs:ce])

            dma_dt(0, nwarm)
            for j in WARM_J_ORDER:
                dma_wt(j)
            dma_dt(nwarm, B)

            # ---- PE p-state warmup: harmless matmuls on a zeroed tile
            # while the first DMAs land, so real matmuls start at full
            # clock.  Reuses the psum pool rotation (no extra banks).
            if WARMUP:
                wz = sb.tile([PE_TILE, 2, 512], f8, tag="wz", name="wz")
                nc.gpsimd.memset(wz[:], 0.0)
                pw = psm.tile([PE_TILE, CHUNK], f32, tag="ps", name="pw")
                for _ in range(WARMUP):
                    nc.tensor.matmul(pw[:, 0:512], wz[:, :, 0:128], wz[:],
                                     start=True, stop=True,
                                     perf_mode=PM.DoubleRow)

            # per-pair fp16 stage-1 results [128, 2, 250, 16]
            def s1_tile():
                return sb.tile([PE_TILE, 2, C_SHARD, 16], f16, tag="s1",
                               bufs=5, name="s1")

            def matmul_chunk(ps, m, j):
                cs, ce, _ = chunks[j]
                w = ce - cs
                for h0 in range(0, w, 512):
                    h1 = min(h0 + 512, w)
                    for kp in range(2):
                        nc.tensor.matmul(
                            ps[:, h0:h1],
                            dt[kp][:, :, m * PE_TILE:(m + 1) * PE_TILE],
                            wt[kp][:, :, cs + h0:cs + h1],
                            start=(kp == 0), stop=(kp == 1),
                            perf_mode=PM.DoubleRow)

            def stage1(ps, s1, osb, t, j, eng):
                cs, ce, ncls = chunks[j]
                c0 = cs // PROXIES
                dst = s1[:, t, c0:c0 + ncls, :]
                ps3 = ps[:, :ce - cs].rearrange("p (c g) -> p c g",
                                                g=PROXIES)
                if eng == 'R':
                    nc.vector.tensor_reduce(osb[:, t, c0:c0 + ncls], ps3,
                                            axis=AX.X, op=OP.max)
                elif eng == 'H':
                    sh = sb.tile([PE_TILE, CHUNK // 2], f16, tag="sh",
                                 bufs=8, name="sh")
                    sh3 = sh[:, :ncls * 16].rearrange("p (c g) -> p c g",
                                                      g=16)
                    nc.scalar.activation(sh3, ps3[:, :, 16:32], AF.Abs)
                    nc.vector.tensor_tensor(dst, ps3[:, :, 0:16], sh3,
                                            op=OP.add)
                elif eng == 'G':
                    sg = sb.tile([PE_TILE, 2, CHUNK // 2], f16, tag="sg",
                                 bufs=12, name="sg")
                    sg_s = sg[:, 0, :ncls * 16].rearrange(
                        "p (c g) -> p c g", g=16)
                    sg_d = sg[:, 1, :ncls * 16].rearrange(
                        "p (c g) -> p c g", g=16)
                    nc.scalar.copy(sg_s, ps3[:, :, 0:16])
                    nc.scalar.activation(sg_d, ps3[:, :, 16:32], AF.Abs)
                    nc.gpsimd.tensor_tensor(dst, sg_s, sg_d, op=OP.add)
                else:
                    raise ValueError(eng)

            def smalls(s1, osb, pr):
                """fp16 tree 16->1 over classes [0, TREE_C) for one pair."""
                kind = SMALLS_PAT[pr]
                e2 = nc.vector if kind in ('D', 'M') else nc.gpsimd
                e = nc.vector if kind == 'D' else nc.gpsimd
                s1v = s1[:, :, 0:TREE_C, :]
                s2 = sb.tile([PE_TILE, 2, TREE_C, 8], f16, tag="s2", bufs=2,
                             name="s2")
                e2.tensor_tensor(s2[:], s1v[:, :, :, 0:8], s1v[:, :, :, 8:16],
                                 op=OP.max)
                s3 = sb.tile([PE_TILE, 2, TREE_C, 4], f16, tag="s3", bufs=2,
                             name="s3")
                e.tensor_tensor(s3[:], s2[:, :, :, 0:4], s2[:, :, :, 4:8],
                                op=OP.max)
                s4 = sb.tile([PE_TILE, 2, TREE_C, 2], f16, tag="s4", bufs=2,
                             name="s4")
                e.tensor_tensor(s4[:], s3[:, :, :, 0:2], s3[:, :, :, 2:4],
                                op=OP.max)
                e.tensor_tensor(osb[:, :, 0:TREE_C], s4[:, :, :, 0],
                                s4[:, :, :, 1], op=OP.max)
                dst = out_d[pr * 2 * PE_TILE:(pr + 1) * 2 * PE_TILE,
                            :].rearrange("(t p) c -> p t c", t=2)
                nc.sync.dma_start(dst, osb[:])

            def smalls_t(s1, osb, pr, t):
                """3D tree for one m-tile (tail pair: overlap the halves)."""
                e = nc.vector
                s1v = s1[:, t, 0:TREE_C, :]
                u2 = sb.tile([PE_TILE, TREE_C, 8], f16, tag="u2", bufs=2,
                             name="u2")
                e.tensor_tensor(u2[:], s1v[:, :, 0:8], s1v[:, :, 8:16],
                                op=OP.max)
                u3 = sb.tile([PE_TILE, TREE_C, 4], f16, tag="u3", bufs=2,
                             name="u3")
                e.tensor_tensor(u3[:], u2[:, :, 0:4], u2[:, :, 4:8],
                                op=OP.max)
                u4 = sb.tile([PE_TILE, TREE_C, 2], f16, tag="u4", bufs=2,
                             name="u4")
                e.tensor_tensor(u4[:], u3[:, :, 0:2], u3[:, :, 2:4],
                                op=OP.max)
                e.tensor_tensor(osb[:, t, 0:TREE_C], u4[:, :, 0],
                                u4[:, :, 1], op=OP.max)
                m = pr * 2 + t
                nc.sync.dma_start(
                    out_d[m * PE_TILE:(m + 1) * PE_TILE, :], osb[:, t])

            def osb_tile():
                return sb.tile([PE_TILE, 2, C_SHARD], f16, tag="osb", bufs=3,
                               name="osb")

            def pair_chunks(pr, s1, osb, mid=None):
                for t in range(2):
                    for j in J_ORDER:
                        ps = psm.tile([PE_TILE, CHUNK], f32, tag="ps",
                                      name="ps")
                        matmul_chunk(ps, pr * 2 + t, j)
                        stage1(ps, s1, osb, t, j, chunk_eng(j))
                    if t == 0 and mid is not None:
                        mid()

            # ---- warm pairs: chunk-outer so the first w DMAs gate only
            # the first chunk; both pairs reuse each chunk while later
            # w/data DMAs stream in.
            s1t = [s1_tile() for _ in range(WARM_PAIRS)]
            obt = [osb_tile() for _ in range(WARM_PAIRS)]
            for j in WARM_J_ORDER:
                for pr in range(WARM_PAIRS):
                    for t in range(2):
                        ps = psm.tile([PE_TILE, CHUNK], f32, tag="ps",
                                      name="ps")
                        matmul_chunk(ps, pr * 2 + t, j)
                        stage1(ps, s1t[pr], obt[pr], t, j, chunk_eng(j))
            smalls(s1t[0], obt[0], 0)

            # ---- remaining pairs, smalls lagging one pair behind.
            s1prev, obprev = s1t[1], obt[1]
            for pr in range(WARM_PAIRS, NPR):
                s1 = s1_tile()
                ob = osb_tile()
                prev, obp, prev_pr = s1prev, obprev, pr - 1
                if pr < NPR - 1:
                    pair_chunks(pr, s1, ob,
                                mid=lambda: smalls(prev, obp, prev_pr))
                else:
                    def mid_last(s1=s1, ob=ob):
                        smalls(prev, obp, prev_pr)
                        smalls_t(s1, ob, NPR - 1, 0)
                    pair_chunks(pr, s1, ob, mid=mid_last)
                s1prev, obprev = s1, ob
            smalls_t(s1prev, obprev, NPR - 1, 1)

    nc.compile()
    return nc


_NC_CACHE = {}


def _get_nc(key):
    if key not in _NC_CACHE:
        _NC_CACHE[key] = build_bass_kernel()
    return _NC_CACHE[key]


def _pack_dr(x8, F):
    """[F, 512] fp8 -> [128, 2kp*2i*F]: x8T[c, kp*2F+i*F+f] =
    x8[f, kp*256+i*128+c] (DoubleRow k-pair/group interleave)."""
    t = np.ascontiguousarray(x8.T).reshape(2, 2, 128, F)     # [kp, i, c, f]
    t = t.transpose(2, 0, 1, 3).reshape(128, 4 * F)
    return np.ascontiguousarray(t)


def kernel(data, w1, segment_ids=None):
    """Full-input entry point: shards internally across 8 NeuronCores."""
    import ml_dtypes
    from concourse.bass_utils import run_bass_kernel_spmd

    data = np.ascontiguousarray(np.asarray(data), dtype=np.float32)
    w1 = np.ascontiguousarray(np.asarray(w1), dtype=np.float32)
    assert data.shape == (B, E) and w1.shape == (P, E)

    dn = np.maximum(np.linalg.norm(data, axis=1, keepdims=True), EPS)
    wn = np.maximum(np.linalg.norm(w1, axis=1, keepdims=True), EPS)
    d8 = ((data / dn) * S).astype(ml_dtypes.float8_e4m3fn)
    wnf = (w1 / wn) * S

    def shard_w8(i):
        ws = np.array(wnf[i * P_SHARD:(i + 1) * P_SHARD])
        for j in SD_JS:
            r0 = j * CHUNK
            blk = ws[r0:r0 + CHUNK].reshape(-1, 16, 2, E)
            sd = np.concatenate(
                [(blk[:, :, 0] + blk[:, :, 1]) * 0.5,
                 (blk[:, :, 0] - blk[:, :, 1]) * 0.5], axis=1)
            ws[r0:r0 + CHUNK] = sd.reshape(CHUNK, E)
        return _pack_dr(ws.astype(ml_dtypes.float8_e4m3fn), P_SHARD)

    d8p = _pack_dr(d8, B)
    nc = _get_nc("full")
    in_maps = [{"d8": d8p, "w8": shard_w8(i)} for i in range(N_CORES)]
    res = run_bass_kernel_spmd(nc, in_maps, core_ids=list(range(N_CORES)))
    out = np.empty((C, B), dtype=np.float32)
    inv = np.float32(1.0 / (S * S))
    for i in range(N_CORES):
        out[i * C_SHARD:(i + 1) * C_SHARD, :] = \
            res.results[i]["out"].astype(np.float32).T * inv
    return out


# revision 65
# speedup vs baseline: 1.0004x; 1.0004x over previous
"""Trainium2 Bass kernel for nn_ImprintedModel (retrieval_knn).

Computes y[c, b] = max over the 32 proxies p of class c of
    (w1[p] / ||w1[p]||) . (data[b] / ||data[b]||)
for data [4096, 512], w1 [64000, 512] (2000 classes x 32 proxies),
output [2000, 4096] fp32.

Sharding: w1 rows (and hence classes) split across 8 cores (8000 rows =
250 classes per core); data replicated.  Each core computes its 250
output rows for all 4096 batch columns; host concatenates/transposes.

Algorithm (per core):
  Host prep (free wrt device time): l2-normalize data rows and w rows,
  scale by S=16, quantize to fp8 e4m3, and pack both operands
  transposed+interleaved for DoubleRow matmuls:
      x8T[c, kp*2F + i*F + f] = x8[f, kp*256 + i*128 + c]
  so a [128, 2, F] SBUF tile per contraction k-pair kp holds the two
  128-row contraction groups the PE consumes per DoubleRow pass.

  Device: for each batch m-tile (128 rows) and each chunk of 32 classes
  (1024 w rows; tail 26/832), four fp8 DoubleRow matmuls (contraction
  512 = 2 k-pairs x 256, two 512-column groups) accumulate scaled
  scores into a [128, 1024] PSUM tile at 0.5 cycles/row -- 4x the
  bf16/f32r rate, ~110us of PE time total.

  The per-class max over 32 proxies is bound by PSUM-drain legality
  (walrus-verified): GPSIMD/Pool cannot access PSUM and has no max op
  (add/copy only), DMA cannot access PSUM, and a DVE tensor_tensor may
  read at most one PSUM operand.  PSUM is drained by DVE and ACT only;
  Pool is recruited through a host-side transform: chunks j0..j4 store
  each proxy pair as s=(w0+w1)/2, d=(w0-w1)/2 rows, so the pairwise max
  becomes s+|d| -- an ADD.  Per chunk: 'G' (j0-3) ACT copies the s-half
  and writes |d| (Abs), then Pool does the fp16 ADD; 'H' (j4) ACT
  writes |d| and DVE adds it to the PSUM s-half; 'R' (j5-7, raw) one
  DVE tensor_reduce(max) finishes all 32 proxies straight from PSUM
  into the output tile.  Chunks are processed in an interleaved order
  (R/H spaced by Gs) so consecutive PSUM tiles drain on different
  engines; a DVE 2x-mode fp16 max tree finishes classes 0..159 per
  m-tile pair, one pair behind the matmul stream (per-half for the
  last pair to shorten the tail).  Scores are 256x true values (S^2);
  the host divides after the gather.
"""

import numpy as np

# Problem shapes (hardcoded; harness always calls with these).
B = 4096
E = 512
C = 2000
PROXIES = 32
P = C * PROXIES
N_CORES = 8
P_SHARD = P // N_CORES      # 8000 w rows per core
C_SHARD = C // N_CORES      # 250 classes per core
EPS = 1e-12
S = 16.0                    # fp8 pre-quant scale (output is S^2 too big)

PE_TILE = 128
MT = B // PE_TILE           # 32 batch m-tiles
NPR = MT // 2               # 16 m-tile pairs
WARM_PAIRS = 2              # leading pairs run chunk-outer (DMA warmup)
CHUNK = 1024                # w rows per chunk (32 classes)
NCH = (P_SHARD + CHUNK - 1) // CHUNK        # 8 chunks (last 832)

# Legal engine facts on TRN2 (walrus-verified): GPSIMD/Pool cannot
# touch PSUM and has no max op at all (add/copy only); DMA cannot touch
# PSUM; a DVE tensor_tensor may read at most one PSUM operand.  PSUM is
# therefore drained by DVE and ACT only.  To still use Pool, chunks
# j0..j4 are stored TRANSFORMED on the host: each proxy pair (2i,2i+1)
# becomes s=(w0+w1)/2, d=(w0-w1)/2 rows, so the pairwise max is s+|d|
# -- an ADD, which Pool does support.
#  'G' (j0..j3): ACT copies s-half + ACT |d|-half -> Pool fp16 ADD
#  'H' (j4):     ACT |d|-half -> DVE ADD (PSUM s-half + SBUF |d|)
#  'R' (j5..j7): DVE tensor_reduce(max) of the raw 32 proxies straight
#                from PSUM into the output tile
# Chunks are processed in an interleaved order so consecutive PSUM
# tiles are drained by different engines (R/H spaced out by Gs).
SD_JS = (0, 1, 2, 3, 4)             # host-transformed chunks
J_ORDER = (0, 5, 1, 6, 2, 4, 3, 7)
WARM_J_ORDER = J_ORDER
TREE_C = 5 * (CHUNK // PROXIES)     # classes finished by the fp16 tree
SMALLS_PAT = ['D'] * 16


def chunk_eng(j):
    if j < 4:
        return 'G'
    if j == 4:
        return 'H'
    return 'R'


WARMUP = 0                 # PE p-state warmup matmuls (0 to disable)


def build_bass_kernel():
    from concourse import bacc, mybir
    from concourse.tile import TileContext

    f32 = mybir.dt.float32
    f16 = mybir.dt.float16
    f8 = mybir.dt.float8e4
    OP = mybir.AluOpType
    AF = mybir.ActivationFunctionType
    AX = mybir.AxisListType
    PM = mybir.MatmulPerfMode

    nc = bacc.Bacc("TRN2", target_bir_lowering=False, debug=False)
    d8_d = nc.dram_tensor("d8", [PE_TILE, 4 * B], f8, kind="ExternalInput")
    w8_d = nc.dram_tensor("w8", [PE_TILE, 4 * P_SHARD], f8,
                          kind="ExternalInput")
    out_d = nc.dram_tensor("out", [B, C_SHARD], f16, kind="ExternalOutput")

    # chunk column ranges and class counts
    chunks = []
    for j in range(NCH):
        cs = j * CHUNK
        ce = min(cs + CHUNK, P_SHARD)
        chunks.append((cs, ce, (ce - cs) // PROXIES))

    with TileContext(nc) as tc:
        with tc.tile_pool(name="sbuf", bufs=1) as sb, \
             tc.tile_pool(name="mmps", bufs=4, space="PSUM") as psm:

            dt = [sb.tile([PE_TILE, 2, B], f8, tag=f"dt{kp}", name=f"dt{kp}")
                  for kp in range(2)]
            wt = [sb.tile([PE_TILE, 2, P_SHARD], f8, tag=f"wt{kp}",
                          name=f"wt{kp}") for kp in range(2)]

            # ---- input DMAs.  Emission order keeps the startup path
            # short: data columns for the warm pairs, then w chunk by
            # chunk, then the remaining data columns.
            nwarm = WARM_PAIRS * 2 * PE_TILE

            def dma_dt(b0, b1):
                for kp in range(2):
                    src = d8_d[:].rearrange("p (k i b) -> p k i b", k=2, i=2)
                    nc.sync.dma_start(dt[kp][:, :, b0:b1],
                                      src[:, kp, :, b0:b1])

            def dma_wt(j):
                cs, ce, _ = chunks[j]
                for kp in range(2):
                    src = w8_d[:].rearrange("p (k i n) -> p k i n", k=2, i=2)
                    nc.sync.dma_start(wt[kp][:, :, cs:ce],
                                      src[:, kp, :, cs:ce])

            dma_dt(0, nwarm)
            for j in WARM_J_ORDER:
                dma_wt(j)
            dma_dt(nwarm, B)

            # ---- PE p-state warmup: harmless matmuls on a zeroed tile
            # while the first DMAs land, so real matmuls start at full
            # clock.  Reuses the psum pool rotation (no extra banks).
            if WARMUP:
                wz = sb.tile([PE_TILE, 2, 512], f8, tag="wz", name="wz")
                nc.gpsimd.memset(wz[:], 0.0)
                pw = psm.tile([PE_TILE, CHUNK], f32, tag="ps", name="pw")
                for _ in range(WARMUP):
                    nc.tensor.matmul(pw[:, 0:512], wz[:, :, 0:128], wz[:],
                                     start=True, stop=True,
                                     perf_mode=PM.DoubleRow)

            # per-pair fp16 stage-1 results [128, 2, 250, 16]
            def s1_tile():
                return sb.tile([PE_TILE, 2, C_SHARD, 16], f16, tag="s1",
                               bufs=5, name="s1")

            def matmul_chunk(ps, m, j):
                cs, ce, _ = chunks[j]
                w = ce - cs
                for h0 in range(0, w, 512):
                    h1 = min(h0 + 512, w)
                    for kp in range(2):
                        nc.tensor.matmul(
                            ps[:, h0:h1],
                            dt[kp][:, :, m * PE_TILE:(m + 1) * PE_TILE],
                            wt[kp][:, :, cs + h0:cs + h1],
                            start=(kp == 0), stop=(kp == 1),
                            perf_mode=PM.DoubleRow)

            def stage1(ps, s1, osb, t, j, eng):
                cs, ce, ncls = chunks[j]
                c0 = cs // PROXIES
                dst = s1[:, t, c0:c0 + ncls, :]
                ps3 = ps[:, :ce - cs].rearrange("p (c g) -> p c g",
                                                g=PROXIES)
                if eng == 'R':
                    nc.vector.tensor_reduce(osb[:, t, c0:c0 + ncls], ps3,
                                            axis=AX.X, op=OP.max)
                elif eng == 'H':
                    sh = sb.tile([PE_TILE, CHUNK // 2], f16, tag="sh",
                                 bufs=8, name="sh")
                    sh3 = sh[:, :ncls * 16].rearrange("p (c g) -> p c g",
                                                      g=16)
                    nc.scalar.activation(sh3, ps3[:, :, 16:32], AF.Abs)
                    nc.vector.tensor_tensor(dst, ps3[:, :, 0:16], sh3,
                                            op=OP.add)
                elif eng == 'G':
                    sg = sb.tile([PE_TILE, 2, CHUNK // 2], f16, tag="sg",
                                 bufs=12, name="sg")
                    sg_s = sg[:, 0, :ncls * 16].rearrange(
                        "p (c g) -> p c g", g=16)
                    sg_d = sg[:, 1, :ncls * 16].rearrange(
                        "p (c g) -> p c g", g=16)
                    nc.scalar.copy(sg_s, ps3[:, :, 0:16])
                    nc.scalar.activation(sg_d, ps3[:, :, 16:32], AF.Abs)
                    nc.gpsimd.tensor_tensor(dst, sg_s, sg_d, op=OP.add)
                else:
                    raise ValueError(eng)

            def smalls(s1, osb, pr):
                """fp16 tree 16->1 over classes [0, TREE_C) for one pair."""
                kind = SMALLS_PAT[pr]
                e2 = nc.vector if kind in ('D', 'M') else nc.gpsimd
                e = nc.vector if kind == 'D' else nc.gpsimd
                s1v = s1[:, :, 0:TREE_C, :]
                s2 = sb.tile([PE_TILE, 2, TREE_C, 8], f16, tag="s2", bufs=2,
                             name="s2")
                e2.tensor_tensor(s2[:], s1v[:, :, :, 0:8], s1v[:, :, :, 8:16],
                                 op=OP.max)
                s3 = sb.tile([PE_TILE, 2, TREE_C, 4], f16, tag="s3", bufs=2,
                             name="s3")
                e.tensor_tensor(s3[:], s2[:, :, :, 0:4], s2[:, :, :, 4:8],
                                op=OP.max)
                s4 = sb.tile([PE_TILE, 2, TREE_C, 2], f16, tag="s4", bufs=2,
                             name="s4")
                e.tensor_tensor(s4[:], s3[:, :, :, 0:2], s3[:, :, :, 2:4],
                                op=OP.max)
                e.tensor_tensor(osb[:, :, 0:TREE_C], s4[:, :, :, 0],
                                s4[:, :, :, 1], op=OP.max)
                dst = out_d[pr * 2 * PE_TILE:(pr + 1) * 2 * PE_TILE,
                            :].rearrange("(t p) c -> p t c", t=2)
                nc.sync.dma_start(dst, osb[:])

            def smalls_t(s1, osb, pr, t):
                """3D tree for one m-tile (tail pair: overlap the halves)."""
                e = nc.vector
                s1v = s1[:, t, 0:TREE_C, :]
                u2 = sb.tile([PE_TILE, TREE_C, 8], f16, tag="u2", bufs=2,
                             name="u2")
                e.tensor_tensor(u2[:], s1v[:, :, 0:8], s1v[:, :, 8:16],
                                op=OP.max)
                u3 = sb.tile([PE_TILE, TREE_C, 4], f16, tag="u3", bufs=2,
                             name="u3")
                e.tensor_tensor(u3[:], u2[:, :, 0:4], u2[:, :, 4:8],
                                op=OP.max)
                u4 = sb.tile([PE_TILE, TREE_C, 2], f16, tag="u4", bufs=2,
                             name="u4")
                e.tensor_tensor(u4[:], u3[:, :, 0:2], u3[:, :, 2:4],
                                op=OP.max)
                e.tensor_tensor(osb[:, t, 0:TREE_C], u4[:, :, 0],
                                u4[:, :, 1], op=OP.max)
                m = pr * 2 + t
                nc.sync.dma_start(
                    out_d[m * PE_TILE:(m + 1) * PE_TILE, :], osb[:, t])

            def osb_tile():
                return sb.tile([PE_TILE, 2, C_SHARD], f16, tag="osb", bufs=3,
                               name="osb")

            def pair_chunks(pr, s1, osb, mid=None):
                for t in range(2):
                    for j in J_ORDER:
                        ps = psm.tile([PE_TILE, CHUNK], f32, tag="ps",
                                      name="ps")
                        matmul_chunk(ps, pr * 2 + t, j)
                        stage1(ps, s1, osb, t, j, chunk_eng(j))
                    if t == 0 and mid is not None:
                        mid()

            # ---- warm pairs: chunk-outer so the first w DMAs gate only
            # the first chunk; both pairs reuse each chunk while later
            # w/data DMAs stream in.
            s1t = [s1_tile() for _ in range(WARM_PAIRS)]
            obt = [osb_tile() for _ in range(WARM_PAIRS)]
            for j in WARM_J_ORDER:
                for pr in range(WARM_PAIRS):
                    for t in range(2):
                        ps = psm.tile([PE_TILE, CHUNK], f32, tag="ps",
                                      name="ps")
                        matmul_chunk(ps, pr * 2 + t, j)
                        stage1(ps, s1t[pr], obt[pr], t, j, chunk_eng(j))
            smalls(s1t[0], obt[0], 0)

            # ---- remaining pairs, smalls lagging one pair behind.
            s1prev, obprev = s1t[1], obt[1]
            for pr in range(WARM_PAIRS, NPR):
                s1 = s1_tile()
                ob = osb_tile()
                prev, obp, prev_pr = s1prev, obprev, pr - 1
                if pr < NPR - 1:
                    pair_chunks(pr, s1, ob,
                                mid=lambda: smalls(prev, obp, prev_pr))
                else:
                    def mid_last(s1=s1, ob=ob):
                        smalls(prev, obp, prev_pr)
                        smalls_t(s1, ob, NPR - 1, 0)
                    pair_chunks(pr, s1, ob, mid=mid_last)
                s1prev, obprev = s1, ob
            smalls_t(s1prev, obprev, NPR - 1, 1)

    nc.compile()
    return nc


_NC_CACHE = {}


def _get_nc(key):
    if key not in _NC_CACHE:
        _NC_CACHE[key] = build_bass_kernel()
    return _NC_CACHE[key]


def _pack_dr(x8, F):
    """[F, 512] fp8 -> [128, 2kp*2i*F]: x8T[c, kp*2F+i*F+f] =
    x8[f, kp*256+i*128+c] (DoubleRow k-pair/group interleave)."""
    t = np.ascontiguousarray(x8.T).reshape(2, 2, 128, F)     # [kp, i, c, f]
    t = t.transpose(2, 0, 1, 3).reshape(128, 4 * F)
    return np.ascontiguousarray(t)


def kernel(data, w1, segment_ids=None):
    """Full-input entry point: shards internally across 8 NeuronCores."""
    import ml_dtypes
    from concourse.bass_utils import run_bass_kernel_spmd

    data = np.ascontiguousarray(np.asarray(data), dtype=np.float32)
    w1 = np.ascontiguousarray(np.asarray(w1), dtype=np.float32)
    assert data.shape == (B, E) and w1.shape == (P, E)

    dn = np.maximum(np.linalg.norm(data, axis=1, keepdims=True), EPS)
    wn = np.maximum(np.linalg.norm(w1, axis=1, keepdims=True), EPS)
    d8 = ((data / dn) * S).astype(ml_dtypes.float8_e4m3fn)
    wnf = (w1 / wn) * S

    def shard_w8(i):
        ws = np.array(wnf[i * P_SHARD:(i + 1) * P_SHARD])
        for j in SD_JS:
            r0 = j * CHUNK
            blk = ws[r0:r0 + CHUNK].reshape(-1, 16, 2, E)
            sd = np.concatenate(
                [(blk[:, :, 0] + blk[:, :, 1]) * 0.5,
                 (blk[:, :, 0] - blk[:, :, 1]) * 0.5], axis=1)
            ws[r0:r0 + CHUNK] = sd.reshape(CHUNK, E)
        return _pack_dr(ws.astype(ml_dtypes.float8_e4m3fn), P_SHARD)

    d8p = _pack_dr(d8, B)
    nc = _get_nc("full")
    in_maps = [{"d8": d8p, "w8": shard_w8(i)} for i in range(N_CORES)]
    res = run_bass_kernel_spmd(nc, in_maps, core_ids=list(range(N_CORES)))
    out = np.empty((C, B), dtype=np.float32)
    inv = np.float32(1.0 / (S * S))
    for i in range(N_CORES):
        out[i * C_SHARD:(i + 1) * C_SHARD, :] = \
            res.results[i]["out"].astype(np.float32).T * inv
    return out
